# revision 74
# baseline (speedup 1.0000x reference)
"""ClusterAggregator Trainium2 kernel.

Computes, per batch element b (one NeuronCore each, 8 cores total):
    h   = relu(F @ W1 + b1)            F: [N, 128]
    imp = sigmoid(h @ W2 + b2)         imp: [N]
    per-cluster softmax(imp) weighted sum of F -> out [C, 128]

Design notes (per core):
  - token t <-> (partition p, tile j) with t = p*J + j, J = N/128 tiles.
  - The HOST uploads features in BOTH layouts as bf16 (no on-device
    transposes, no cast-DMAs — those dominated the previous version):
      featp [p, j, d+1]  t-major, with a baked-in ones column at d=D
                         (makes the softmax denominator fall out of the
                         segment matmul); contiguous 66 KB per partition.
      featt [d, j, p]    d-major for the MLP lhsT tiles; contiguous
                         64 KB per partition.
    Total HBM read ~16.9 MB/core of pure contiguous HWDGE loads.
  - W2 is folded into W1 host-side: W1s[:, k] = W1[:, perm[k]] * |w2[perm[k]]|
    with positive-w2 columns first (mp of them). Then
        z = sum(relu(h*[:, :mp])) - sum(relu(h*[:, mp:]))  (+ b2 handled later)
    which replaces the second linear layer with two strided reduces.
  - sigmoid via tanh (same ACT table set as exp => zero table switches):
        sigmoid(z + b2) = 0.5 + 0.5*tanh(0.5*z + 0.5*b2)
        e = exp(sigmoid) = exp(0.5*t + 0.5)
  - scaled one-hot built per BLK-tile block in two batched DVE ops
    (is_equal against an uploaded iota_rep [p, c, jj], then mult by a
    broadcast e) in [P, C, BLK] layout so all broadcasts are mid-dim.
  - per tile j: MLP matmul  h[t,H] = FT[:,j,:]^T @ W1s   (PSUM, 2 banks/blk)
               seg matmul  seg[C,D+1] += pe[:,:,jj]^T @ featp tile
    with the seg matmuls of block b issued after the MLP matmuls of
    block b+1 so PE never stalls on the DVE/ACT chain.
  - Final out = numer / max(denom, tiny).
"""

import os
import sys

sys.path.insert(0, "/opt/trn_rl_repo")

from contextlib import ExitStack

import ml_dtypes
import numpy as np

def _install_axon_hooks_shim():
    """The agent image's antenv lacks axon_hooks; recreate the NTFF profile
    hook (a (dir, device_ids) -> contextmanager driving libaxon_pjrt.so)
    so run_bass_kernel_spmd(trace=True) works under axon."""
    import contextlib
    import ctypes
    import types

    if "antenv.axon_hooks" in sys.modules:
        return
    mod = types.ModuleType("antenv.axon_hooks")
    _state = {"hook": None}

    so_path = "/opt/axon/libaxon_pjrt.so"
    hook = None
    if os.path.exists(so_path):
        lib = ctypes.CDLL(so_path)
        if hasattr(lib, "axon_start_nrt_profile"):
            lib.axon_start_nrt_profile.argtypes = [
                ctypes.POINTER(ctypes.c_int64),
                ctypes.c_size_t,
            ]
            lib.axon_start_nrt_profile.restype = ctypes.c_int64
            lib.axon_stop_nrt_profile.argtypes = [ctypes.c_char_p]
            lib.axon_stop_nrt_profile.restype = ctypes.c_int64

            @contextlib.contextmanager
            def _hook(output_dir, device_ids):
                import jax

                jax.devices()
                if device_ids:
                    ids = (ctypes.c_int64 * len(device_ids))(*device_ids)
                    rc = lib.axon_start_nrt_profile(ids, len(device_ids))
                else:
                    rc = lib.axon_start_nrt_profile(None, 0)
                if rc != 0:
                    raise RuntimeError(f"axon_start_nrt_profile rc={rc}")
                try:
                    yield
                finally:
                    n = lib.axon_stop_nrt_profile(str(output_dir).encode())
                    if n < 0:
                        raise RuntimeError(f"axon_stop_nrt_profile rc={n}")
                    print(f"profile: {n} file(s) written to {output_dir}")

            hook = _hook
    _state["hook"] = hook

    mod.set_axon_ntff_profile_hook = lambda h: _state.__setitem__("hook", h)
    mod.get_axon_ntff_profile_hook = lambda: _state["hook"]
    sys.modules["antenv.axon_hooks"] = mod


_install_axon_hooks_shim()

import concourse.bass as bass
import concourse.tile as tile
from concourse import bacc, mybir
from concourse.bass_utils import run_bass_kernel_spmd

BF16 = mybir.dt.bfloat16
F32 = mybir.dt.float32
F8 = mybir.dt.float8e4
BF16_NP = ml_dtypes.bfloat16
F8_NP = ml_dtypes.float8_e4m3

P = 128          # partitions / tokens per tile
D = 128          # feature dim
C = 64           # clusters
H = 64           # hidden dim
BLK = 16         # tiles per pipeline block (one PSUM 2-bank group)
CH = 32          # tiles per DMA chunk (~1.05 MB each, 8.2 KB/partition)

# "vector" (default) or "gpsimd": engine for the scaled-one-hot is_equal.
# (gpsimd fails walrus codegen for broadcast-AP tensor_tensor.)
PE_ENGINE = os.environ.get("KERNEL_PE_ENGINE", "vector")
# fp8(e4m3) featt for the MLP path (halves that tensor's HBM traffic).
USE_FP8 = os.environ.get("KERNEL_FP8", "1") == "1"
# fp8(e4m3) featp for the segment-sum path as well — measured rel err
# 2.6e-2 (> the 2e-2 gate): the quantized features feed the output sum
# directly, so keep this OFF.
USE_FP8P = os.environ.get("KERNEL_FP8P", "0") == "1"
# run even/odd segment-matmul chains concurrently in PE column groups.
SEG2 = os.environ.get("KERNEL_SEG2", "1") == "1"

LAST_RESULTS = None  # BassKernelResults of the most recent kernel() call


def _build_program(N: int, mp: int, b2: float, b1_nonzero: bool):
    """One SPMD program; every core runs it on its own batch element."""
    J = N // P            # token tiles
    assert N % P == 0 and J % (2 * BLK) == 0 and J % CH == 0

    nc = bacc.Bacc(
        "TRN2",
        target_bir_lowering=False,
        debug=False,
        enable_asserts=False,
        num_devices=8,
    )

    FT_DT = F8 if USE_FP8 else BF16
    FP_DT = F8 if USE_FP8P else BF16
    nblk = J // BLK
    featp = nc.dram_tensor("featp", [P, J * (D + 1)], FP_DT, kind="ExternalInput")
    featt = nc.dram_tensor("featt", [D, J * P], FT_DT, kind="ExternalInput")
    a_bf = nc.dram_tensor("a_bf", [P, J], BF16, kind="ExternalInput")
    iota_rep = nc.dram_tensor(
        "iota_rep", [P, C * 2 * BLK], BF16, kind="ExternalInput"
    )
    w1s = nc.dram_tensor("w1s", [D, H], BF16, kind="ExternalInput")
    b1s = nc.dram_tensor("b1s", [1, H], BF16, kind="ExternalInput")
    # raw even/odd partial segment sums (numer | denom); combined on host
    out_rows = 2 * C if SEG2 else C
    out = nc.dram_tensor("out", [out_rows, D + 1], F32, kind="ExternalOutput")

    with tile.TileContext(nc) as tc, ExitStack() as ctx:
        const_pool = ctx.enter_context(tc.tile_pool(name="consts", bufs=1))
        f1pool = ctx.enter_context(tc.tile_pool(name="f1", bufs=1))
        ftpool = ctx.enter_context(tc.tile_pool(name="ft", bufs=1))
        rhpool = ctx.enter_context(tc.tile_pool(name="rh", bufs=3))
        cmpool = ctx.enter_context(tc.tile_pool(name="cmp", bufs=J // (2 * BLK)))
        zpool = ctx.enter_context(tc.tile_pool(name="zz", bufs=4))
        epool = ctx.enter_context(tc.tile_pool(name="e", bufs=4))
        pepool = ctx.enter_context(tc.tile_pool(name="pe", bufs=4))
        opool = ctx.enter_context(tc.tile_pool(name="outp", bufs=1))
        hpsum = ctx.enter_context(tc.tile_pool(name="hps", bufs=3, space="PSUM"))
        spsum = ctx.enter_context(tc.tile_pool(name="sps", bufs=1, space="PSUM"))

        # ---- constants / small loads (w1s first: it gates the first MM) ----
        w1s_sb = const_pool.tile([D, H], BF16)
        nc.sync.dma_start(w1s_sb[:], w1s.ap())
        a_sb = const_pool.tile([P, J], BF16)
        iota2_sb = const_pool.tile([P, C, 2 * BLK], BF16)
        bias_t = const_pool.tile([P, 1], F32)
        nc.vector.memset(bias_t[:], float(0.5 * b2))
        bias_e = const_pool.tile([P, 1], F32)
        nc.vector.memset(bias_e[:], 0.5)
        if b1_nonzero:
            ones1 = const_pool.tile([1, P], BF16)
            nc.vector.memset(ones1[:], 1.0)
            b1s_sb = const_pool.tile([1, H], BF16)
            nc.sync.dma_start(b1s_sb[:], b1s.ap())

        # ---- resident bf16 features in both layouts ----
        F1 = f1pool.tile([P, J, D + 1], FP_DT)  # [token-lane, tile, d | ones]
        FT = ftpool.tile([P, J, D], FT_DT)      # [d, tile, token-lane]
        featp_r = featp.ap().rearrange("p (j d) -> p j d", j=J)
        featt_r = featt.ap().rearrange("d (j t) -> d j t", j=J)
        # Both feature tensors stream through the gpsimd SWDGE queue,
        # interleaved featt/featp so tile ranges arrive in compute order.
        # (Measured: the SWDGE queue sustains ~330-380 GB/s here while the
        # HWDGE rings crawl at ~100-180 GB/s for the same transfers.)
        # a/iota first on the fast queue: they gate the hoisted one-hot
        # compares that fill DVE's head idle time.
        nc.gpsimd.dma_start(a_sb[:], a_bf.ap())
        nc.gpsimd.dma_start(
            iota2_sb[:], iota_rep.ap().rearrange("p (c b) -> p c b", c=C)
        )
        for c0 in range(0, J, CH):
            nc.gpsimd.dma_start(FT[:, c0 : c0 + CH, :], featt_r[:, c0 : c0 + CH, :])
            nc.gpsimd.dma_start(F1[:, c0 : c0 + CH, :], featp_r[:, c0 : c0 + CH, :])

        # ---- persistent segment-sum accumulator ----
        # With SEG2, even tiles accumulate on PSUM partitions 0:64 via PE
        # column group 0 and odd tiles on partitions 64:128 via group 2, so
        # consecutive seg matmuls stream concurrently; host sums the halves.
        seg = spsum.tile([out_rows, D + 1], F32, name="seg")
        pe_tiles: dict[int, object] = {}

        def emit_seg(b):
            pe = pe_tiles.pop(b)
            for jj in range(BLK):
                j = b * BLK + jj
                if SEG2:
                    odd = j % 2
                    nc.tensor.matmul(
                        seg[odd * C : (odd + 1) * C, :],
                        lhsT=pe[:, :, jj],
                        rhs=F1[:, j, :],
                        start=(j < 2),
                        stop=(j >= J - 2),
                        tile_position=(0, odd * C),
                    )
                else:
                    nc.tensor.matmul(
                        seg[:],
                        lhsT=pe[:, :, jj],
                        rhs=F1[:, j, :],
                        start=(j == 0),
                        stop=(j == J - 1),
                    )

        # Even-width reduce splits keep the DVE reduces in packed 2x mode;
        # when mp is odd, column me=mp-1 (positive) lands in the "negative"
        # reduce, so add it back twice via cheap GpSimd fix-up ops.
        me = mp & ~1
        odd_mp = mp != me

        # Hoisted one-hot compares (two blocks per op): they depend only on
        # a/iota, so DVE burns through them while awaiting the first MLP.
        cmp_tiles = []
        for blk2 in range(nblk // 2):
            j0 = blk2 * 2 * BLK
            cmp = cmpool.tile([P, C, 2 * BLK], BF16, name="cmp")
            nc.vector.tensor_tensor(
                cmp[:],
                iota2_sb[:],
                a_sb[:, j0 : j0 + 2 * BLK][:, None, :].broadcast_to(
                    [P, C, 2 * BLK]
                ),
                op=mybir.AluOpType.is_equal,
            )
            cmp_tiles.append(cmp)

        for blk in range(nblk):
            j0 = blk * BLK

            # h* = F @ W1s   [t, H] per tile; one 2-bank PSUM group per block
            hb = hpsum.tile([P, BLK, H], F32)
            for jj in range(BLK):
                nc.tensor.matmul(
                    hb[:, jj, :],
                    lhsT=FT[:, j0 + jj, :],
                    rhs=w1s_sb[:],
                    start=True,
                    stop=not b1_nonzero,
                )
                if b1_nonzero:
                    nc.tensor.matmul(
                        hb[:, jj, :],
                        lhsT=ones1[:],
                        rhs=b1s_sb[:],
                        start=False,
                        stop=True,
                    )

            # segment matmuls run two blocks behind the MLP so PE has dense
            # work while the ACT/DVE chain of the current block runs.
            if blk > 1:
                emit_seg(blk - 2)

            # relu -> bf16 SBUF
            rh = rhpool.tile([P, BLK, H], BF16)
            nc.scalar.activation(rh[:], hb[:], mybir.ActivationFunctionType.Relu)

            # z = sum(pos cols) - sum(neg cols), per (p, jj)
            zz = zpool.tile([P, 2, BLK], F32)
            if me > 0:
                nc.vector.tensor_reduce(
                    zz[:, 0, :], rh[:, :, 0:me],
                    axis=mybir.AxisListType.X, op=mybir.AluOpType.add,
                )
            else:
                nc.vector.memset(zz[:, 0, :], 0.0)
            if me < H:
                nc.vector.tensor_reduce(
                    zz[:, 1, :], rh[:, :, me:H],
                    axis=mybir.AxisListType.X, op=mybir.AluOpType.add,
                )
            else:
                nc.vector.memset(zz[:, 1, :], 0.0)
            s1 = zpool.tile([P, BLK], F32, name="s1")
            if odd_mp:
                sa = zpool.tile([P, BLK], F32, name="sa")
                nc.gpsimd.tensor_tensor(
                    sa[:], zz[:, 0, :], zz[:, 1, :], op=mybir.AluOpType.subtract
                )
                sb_ = zpool.tile([P, BLK], F32, name="sb_")
                nc.gpsimd.tensor_tensor(
                    sb_[:], rh[:, :, me : me + 1], rh[:, :, me : me + 1],
                    op=mybir.AluOpType.add,
                )
                nc.gpsimd.tensor_tensor(
                    s1[:], sa[:], sb_[:], op=mybir.AluOpType.add
                )
            else:
                nc.gpsimd.tensor_tensor(
                    s1[:], zz[:, 0, :], zz[:, 1, :], op=mybir.AluOpType.subtract
                )

            # e = exp(sigmoid(z + b2)) via tanh
            t1 = epool.tile([P, BLK], F32, name="t1")
            nc.scalar.activation(
                t1[:], s1[:], mybir.ActivationFunctionType.Tanh,
                bias=bias_t[:], scale=0.5,
            )
            ee = epool.tile([P, BLK], BF16, name="ee")
            nc.scalar.activation(
                ee[:], t1[:], mybir.ActivationFunctionType.Exp,
                bias=bias_e[:], scale=0.5,
            )

            # scaled one-hot from the hoisted compare, [P, C, BLK] so the
            # e broadcast is mid-dimension
            boff = (blk % 2) * BLK
            pe = pepool.tile([P, C, BLK], BF16)
            nc.vector.tensor_tensor(
                pe[:],
                cmp_tiles[blk // 2][:, :, boff : boff + BLK],
                ee[:][:, None, :].broadcast_to([P, C, BLK]),
                op=mybir.AluOpType.mult,
            )
            pe_tiles[blk] = pe

        emit_seg(nblk - 2)
        emit_seg(nblk - 1)

        # ---- export raw partial sums; host combines + divides ----
        res = opool.tile([out_rows, D + 1], F32)
        nc.vector.tensor_copy(res[:], seg[:])
        nc.gpsimd.dma_start(out.ap(), res[:])

    nc.compile()
    return nc


_PROGRAM_CACHE: dict = {}


def _get_program(N, mp, b2, b1_nonzero):
    key = (N, mp, float(b2), bool(b1_nonzero), PE_ENGINE, USE_FP8, USE_FP8P, SEG2)
    if key not in _PROGRAM_CACHE:
        _PROGRAM_CACHE[key] = _build_program(N, mp, b2, b1_nonzero)
    return _PROGRAM_CACHE[key]


def _host_prep(W1, b1, W2, b2):
    """Fold W2 into W1: scale columns by |w2|, positive-w2 columns first."""
    w2 = np.asarray(W2, np.float32).reshape(-1)
    b1 = np.asarray(b1, np.float32).reshape(-1)
    order = np.argsort(~(w2 >= 0), kind="stable")  # positives first
    mp = int((w2 >= 0).sum())
    w1s = (np.asarray(W1, np.float32)[:, order] * np.abs(w2[order])).astype(BF16_NP)
    b1s = (b1[order] * np.abs(w2[order])).astype(BF16_NP)[None, :]
    b1_nonzero = bool(np.any(b1 != 0))
    return w1s, b1s, mp, float(np.asarray(b2).reshape(-1)[0]), b1_nonzero


def kernel(features, cluster_assignments, W1, b1, W2, b2, num_clusters):
    global LAST_RESULTS
    features = np.asarray(features, np.float32)
    B, N, Din = features.shape
    assert Din == D
    assert int(num_clusters) == C

    w1s, b1s, mp, b2f, b1_nonzero = _host_prep(W1, b1, W2, b2)
    a = np.asarray(cluster_assignments).astype(np.int32)
    iota_np = np.ascontiguousarray(
        np.broadcast_to(
            np.arange(C, dtype=BF16_NP)[None, :, None], (P, C, 2 * BLK)
        )
    ).reshape(P, C * 2 * BLK)

    nc = _get_program(N, mp, b2f, b1_nonzero)

    J = N // P
    ft_np_dt = F8_NP if USE_FP8 else BF16_NP
    fp_np_dt = F8_NP if USE_FP8P else BF16_NP
    in_maps = []
    for b in range(B):
        fp32 = features[b].reshape(P, J, D)  # t = p*J + j
        featp_np = np.empty((P, J, D + 1), fp_np_dt)
        featp_np[:, :, :D] = fp32.astype(fp_np_dt)
        featp_np[:, :, D] = fp_np_dt(1.0)
        featt_np = np.ascontiguousarray(fp32.transpose(2, 1, 0)).astype(ft_np_dt)
        in_maps.append(
            {
                "featp": featp_np.reshape(P, J * (D + 1)),
                "featt": featt_np.reshape(D, J * P),
                "a_bf": a[b].reshape(P, J).astype(BF16_NP),
                "iota_rep": iota_np,
                "w1s": w1s,
                "b1s": b1s,
            }
        )

    res = run_bass_kernel_spmd(nc, in_maps, list(range(B)))
    LAST_RESULTS = res
    raw = np.stack(
        [np.asarray(res.results[i]["out"], np.float64) for i in range(B)], axis=0
    )
    if SEG2:
        raw = raw[:, :C, :] + raw[:, C:, :]
    numer = raw[:, :, :D]
    denom = np.maximum(raw[:, :, D:], 1e-20)
    return (numer / denom).astype(np.float32)


# revision 75
# speedup vs baseline: 1.0414x; 1.0414x over previous
"""ClusterAggregator Trainium2 kernel.

Computes, per batch element b (one NeuronCore each, 8 cores total):
    h   = relu(F @ W1 + b1)            F: [N, 128]
    imp = sigmoid(h @ W2 + b2)         imp: [N]
    per-cluster softmax(imp) weighted sum of F -> out [C, 128]

Design notes (per core):
  - token t <-> (partition p, tile j) with t = p*J + j, J = N/128 tiles.
  - The HOST uploads features in BOTH layouts as bf16 (no on-device
    transposes, no cast-DMAs — those dominated the previous version):
      featp [p, j, d+1]  t-major, with a baked-in ones column at d=D
                         (makes the softmax denominator fall out of the
                         segment matmul); contiguous 66 KB per partition.
      featt [d, j, p]    d-major for the MLP lhsT tiles; contiguous
                         64 KB per partition.
    Total HBM read ~16.9 MB/core of pure contiguous HWDGE loads.
  - W2 is folded into W1 host-side: W1s[:, k] = W1[:, perm[k]] * |w2[perm[k]]|
    with positive-w2 columns first (mp of them). Then
        z = sum(relu(h*[:, :mp])) - sum(relu(h*[:, mp:]))  (+ b2 handled later)
    which replaces the second linear layer with two strided reduces.
  - sigmoid via tanh (same ACT table set as exp => zero table switches):
        sigmoid(z + b2) = 0.5 + 0.5*tanh(0.5*z + 0.5*b2)
        e = exp(sigmoid) = exp(0.5*t + 0.5)
  - scaled one-hot built per BLK-tile block in two batched DVE ops
    (is_equal against an uploaded iota_rep [p, c, jj], then mult by a
    broadcast e) in [P, C, BLK] layout so all broadcasts are mid-dim.
  - per tile j: MLP matmul  h[t,H] = FT[:,j,:]^T @ W1s   (PSUM, 2 banks/blk)
               seg matmul  seg[C,D+1] += pe[:,:,jj]^T @ featp tile
    with the seg matmuls of block b issued after the MLP matmuls of
    block b+1 so PE never stalls on the DVE/ACT chain.
  - Final out = numer / max(denom, tiny).
"""

import os
import sys

sys.path.insert(0, "/opt/trn_rl_repo")

from contextlib import ExitStack

import ml_dtypes
import numpy as np

def _install_axon_hooks_shim():
    """The agent image's antenv lacks axon_hooks; recreate the NTFF profile
    hook (a (dir, device_ids) -> contextmanager driving libaxon_pjrt.so)
    so run_bass_kernel_spmd(trace=True) works under axon."""
    import contextlib
    import ctypes
    import types

    if "antenv.axon_hooks" in sys.modules:
        return
    mod = types.ModuleType("antenv.axon_hooks")
    _state = {"hook": None}

    so_path = "/opt/axon/libaxon_pjrt.so"
    hook = None
    if os.path.exists(so_path):
        lib = ctypes.CDLL(so_path)
        if hasattr(lib, "axon_start_nrt_profile"):
            lib.axon_start_nrt_profile.argtypes = [
                ctypes.POINTER(ctypes.c_int64),
                ctypes.c_size_t,
            ]
            lib.axon_start_nrt_profile.restype = ctypes.c_int64
            lib.axon_stop_nrt_profile.argtypes = [ctypes.c_char_p]
            lib.axon_stop_nrt_profile.restype = ctypes.c_int64

            @contextlib.contextmanager
            def _hook(output_dir, device_ids):
                import jax

                jax.devices()
                if device_ids:
                    ids = (ctypes.c_int64 * len(device_ids))(*device_ids)
                    rc = lib.axon_start_nrt_profile(ids, len(device_ids))
                else:
                    rc = lib.axon_start_nrt_profile(None, 0)
                if rc != 0:
                    raise RuntimeError(f"axon_start_nrt_profile rc={rc}")
                try:
                    yield
                finally:
                    n = lib.axon_stop_nrt_profile(str(output_dir).encode())
                    if n < 0:
                        raise RuntimeError(f"axon_stop_nrt_profile rc={n}")
                    print(f"profile: {n} file(s) written to {output_dir}")

            hook = _hook
    _state["hook"] = hook

    mod.set_axon_ntff_profile_hook = lambda h: _state.__setitem__("hook", h)
    mod.get_axon_ntff_profile_hook = lambda: _state["hook"]
    sys.modules["antenv.axon_hooks"] = mod


_install_axon_hooks_shim()

import concourse.bass as bass
import concourse.tile as tile
from concourse import bacc, mybir
from concourse.bass_utils import run_bass_kernel_spmd

BF16 = mybir.dt.bfloat16
F32 = mybir.dt.float32
F8 = mybir.dt.float8e4
BF16_NP = ml_dtypes.bfloat16
F8_NP = ml_dtypes.float8_e4m3

P = 128          # partitions / tokens per tile
D = 128          # feature dim
C = 64           # clusters
H = 64           # hidden dim
BLK = 16         # tiles per pipeline block (one PSUM 2-bank group)
CH = 32          # tiles per DMA chunk (~1.05 MB each, 8.2 KB/partition)

# "vector" (default) or "gpsimd": engine for the scaled-one-hot is_equal.
# (gpsimd fails walrus codegen for broadcast-AP tensor_tensor.)
PE_ENGINE = os.environ.get("KERNEL_PE_ENGINE", "vector")
# fp8(e4m3) featt for the MLP path (halves that tensor's HBM traffic).
USE_FP8 = os.environ.get("KERNEL_FP8", "1") == "1"
# fp8(e4m3) featp for the segment-sum path as well — measured rel err
# 2.6e-2 (> the 2e-2 gate): the quantized features feed the output sum
# directly, so keep this OFF.
USE_FP8P = os.environ.get("KERNEL_FP8P", "0") == "1"
# run even/odd segment-matmul chains concurrently in PE column groups.
SEG2 = os.environ.get("KERNEL_SEG2", "1") == "1"

LAST_RESULTS = None  # BassKernelResults of the most recent kernel() call


def _build_program(N: int, mp: int, b2: float, b1_nonzero: bool):
    """One SPMD program; every core runs it on its own batch element."""
    J = N // P            # token tiles
    assert N % P == 0 and J % (2 * BLK) == 0 and J % CH == 0

    nc = bacc.Bacc(
        "TRN2",
        target_bir_lowering=False,
        debug=False,
        enable_asserts=False,
        num_devices=8,
    )

    FT_DT = F8 if USE_FP8 else BF16
    FP_DT = F8 if USE_FP8P else BF16
    nblk = J // BLK
    featp = nc.dram_tensor("featp", [P, J * (D + 1)], FP_DT, kind="ExternalInput")
    featt = nc.dram_tensor("featt", [D, J * P], FT_DT, kind="ExternalInput")
    a_bf = nc.dram_tensor("a_bf", [P, J], BF16, kind="ExternalInput")
    iota_rep = nc.dram_tensor(
        "iota_rep", [P, C * 2 * BLK], BF16, kind="ExternalInput"
    )
    w1s = nc.dram_tensor("w1s", [D, H], BF16, kind="ExternalInput")
    b1s = nc.dram_tensor("b1s", [1, H], BF16, kind="ExternalInput")
    # raw even/odd partial segment sums (numer | denom); combined on host
    out_rows = 2 * C if SEG2 else C
    out = nc.dram_tensor("out", [out_rows, D + 1], F32, kind="ExternalOutput")

    with tile.TileContext(nc) as tc, ExitStack() as ctx:
        const_pool = ctx.enter_context(tc.tile_pool(name="consts", bufs=1))
        f1pool = ctx.enter_context(tc.tile_pool(name="f1", bufs=1))
        ftpool = ctx.enter_context(tc.tile_pool(name="ft", bufs=1))
        rhpool = ctx.enter_context(tc.tile_pool(name="rh", bufs=3))
        cmpool = ctx.enter_context(tc.tile_pool(name="cmp", bufs=J // (2 * BLK)))
        zpool = ctx.enter_context(tc.tile_pool(name="zz", bufs=4))
        epool = ctx.enter_context(tc.tile_pool(name="e", bufs=4))
        pepool = ctx.enter_context(tc.tile_pool(name="pe", bufs=4))
        opool = ctx.enter_context(tc.tile_pool(name="outp", bufs=1))
        hpsum = ctx.enter_context(tc.tile_pool(name="hps", bufs=3, space="PSUM"))
        spsum = ctx.enter_context(tc.tile_pool(name="sps", bufs=1, space="PSUM"))

        # ---- constants / small loads (w1s first: it gates the first MM) ----
        w1s_sb = const_pool.tile([D, H], BF16)
        nc.sync.dma_start(w1s_sb[:], w1s.ap())
        a_sb = const_pool.tile([P, J], BF16)
        iota2_sb = const_pool.tile([P, C, 2 * BLK], BF16)
        bias_t = const_pool.tile([P, 1], F32)
        nc.vector.memset(bias_t[:], float(0.5 * b2))
        bias_e = const_pool.tile([P, 1], F32)
        nc.vector.memset(bias_e[:], 0.5)
        if b1_nonzero:
            ones1 = const_pool.tile([1, P], BF16)
            nc.vector.memset(ones1[:], 1.0)
            b1s_sb = const_pool.tile([1, H], BF16)
            nc.sync.dma_start(b1s_sb[:], b1s.ap())

        # ---- resident bf16 features in both layouts ----
        F1 = f1pool.tile([P, J, D + 1], FP_DT)  # [token-lane, tile, d | ones]
        FT = ftpool.tile([P, J, D], FT_DT)      # [d, tile, token-lane]
        featp_r = featp.ap().rearrange("p (j d) -> p j d", j=J)
        featt_r = featt.ap().rearrange("d (j t) -> d j t", j=J)
        # Both feature tensors stream through the gpsimd SWDGE queue,
        # interleaved featt/featp so tile ranges arrive in compute order.
        # (Measured: the SWDGE queue sustains ~330-380 GB/s here while the
        # HWDGE rings crawl at ~100-180 GB/s for the same transfers.)
        # a/iota first on the fast queue: they gate the hoisted one-hot
        # compares that fill DVE's head idle time.
        nc.gpsimd.dma_start(a_sb[:], a_bf.ap())
        nc.gpsimd.dma_start(
            iota2_sb[:], iota_rep.ap().rearrange("p (c b) -> p c b", c=C)
        )
        for c0 in range(0, J, CH):
            nc.gpsimd.dma_start(FT[:, c0 : c0 + CH, :], featt_r[:, c0 : c0 + CH, :])
            nc.gpsimd.dma_start(F1[:, c0 : c0 + CH, :], featp_r[:, c0 : c0 + CH, :])

        # ---- persistent segment-sum accumulator ----
        # With SEG2, even tiles accumulate on PSUM partitions 0:64 via PE
        # column group 0 and odd tiles on partitions 64:128 via group 2, so
        # consecutive seg matmuls stream concurrently; host sums the halves.
        seg = spsum.tile([out_rows, D + 1], F32, name="seg")
        pe_tiles: dict[int, object] = {}

        def emit_seg(b):
            pe = pe_tiles.pop(b)
            for jj in range(BLK):
                j = b * BLK + jj
                if SEG2:
                    odd = j % 2
                    nc.tensor.matmul(
                        seg[odd * C : (odd + 1) * C, :],
                        lhsT=pe[:, :, jj],
                        rhs=F1[:, j, :],
                        start=(j < 2),
                        stop=(j >= J - 2),
                        tile_position=(0, odd * C),
                    )
                else:
                    nc.tensor.matmul(
                        seg[:],
                        lhsT=pe[:, :, jj],
                        rhs=F1[:, j, :],
                        start=(j == 0),
                        stop=(j == J - 1),
                    )

        # Even-width reduce splits keep the DVE reduces in packed 2x mode;
        # when mp is odd, column me=mp-1 (positive) lands in the "negative"
        # reduce, so add it back twice via cheap GpSimd fix-up ops.
        me = mp & ~1
        odd_mp = mp != me

        # Hoisted one-hot compares (two blocks per op): they depend only on
        # a/iota, so DVE burns through them while awaiting the first MLP.
        cmp_tiles = []
        for blk2 in range(nblk // 2):
            j0 = blk2 * 2 * BLK
            cmp = cmpool.tile([P, C, 2 * BLK], BF16, name="cmp")
            nc.vector.tensor_tensor(
                cmp[:],
                iota2_sb[:],
                a_sb[:, j0 : j0 + 2 * BLK][:, None, :].broadcast_to(
                    [P, C, 2 * BLK]
                ),
                op=mybir.AluOpType.is_equal,
            )
            cmp_tiles.append(cmp)

        for blk in range(nblk):
            j0 = blk * BLK

            # h* = F @ W1s   [t, H] per tile; one 2-bank PSUM group per block
            hb = hpsum.tile([P, BLK, H], F32)
            for jj in range(BLK):
                nc.tensor.matmul(
                    hb[:, jj, :],
                    lhsT=FT[:, j0 + jj, :],
                    rhs=w1s_sb[:],
                    start=True,
                    stop=not b1_nonzero,
                )
                if b1_nonzero:
                    nc.tensor.matmul(
                        hb[:, jj, :],
                        lhsT=ones1[:],
                        rhs=b1s_sb[:],
                        start=False,
                        stop=True,
                    )

            # segment matmuls run two blocks behind the MLP so PE has dense
            # work while the ACT/DVE chain of the current block runs.
            if blk > 1:
                emit_seg(blk - 2)

            # relu -> bf16 SBUF
            rh = rhpool.tile([P, BLK, H], BF16)
            nc.scalar.activation(rh[:], hb[:], mybir.ActivationFunctionType.Relu)

            # z = sum(pos cols) - sum(neg cols), per (p, jj)
            zz = zpool.tile([P, 2, BLK], F32)
            if me > 0:
                nc.vector.tensor_reduce(
                    zz[:, 0, :], rh[:, :, 0:me],
                    axis=mybir.AxisListType.X, op=mybir.AluOpType.add,
                )
            else:
                nc.vector.memset(zz[:, 0, :], 0.0)
            if me < H:
                nc.vector.tensor_reduce(
                    zz[:, 1, :], rh[:, :, me:H],
                    axis=mybir.AxisListType.X, op=mybir.AluOpType.add,
                )
            else:
                nc.vector.memset(zz[:, 1, :], 0.0)
            s1 = zpool.tile([P, BLK], F32, name="s1")
            if odd_mp:
                sa = zpool.tile([P, BLK], F32, name="sa")
                nc.gpsimd.tensor_tensor(
                    sa[:], zz[:, 0, :], zz[:, 1, :], op=mybir.AluOpType.subtract
                )
                sb_ = zpool.tile([P, BLK], F32, name="sb_")
                nc.gpsimd.tensor_tensor(
                    sb_[:], rh[:, :, me : me + 1], rh[:, :, me : me + 1],
                    op=mybir.AluOpType.add,
                )
                nc.gpsimd.tensor_tensor(
                    s1[:], sa[:], sb_[:], op=mybir.AluOpType.add
                )
            else:
                nc.gpsimd.tensor_tensor(
                    s1[:], zz[:, 0, :], zz[:, 1, :], op=mybir.AluOpType.subtract
                )

            # e = exp(sigmoid(z + b2)) via tanh
            t1 = epool.tile([P, BLK], F32, name="t1")
            nc.scalar.activation(
                t1[:], s1[:], mybir.ActivationFunctionType.Tanh,
                bias=bias_t[:], scale=0.5,
            )
            ee = epool.tile([P, BLK], BF16, name="ee")
            nc.scalar.activation(
                ee[:], t1[:], mybir.ActivationFunctionType.Exp,
                bias=bias_e[:], scale=0.5,
            )

            # scaled one-hot from the hoisted compare, [P, C, BLK] so the
            # e broadcast is mid-dimension
            boff = (blk % 2) * BLK
            pe = pepool.tile([P, C, BLK], BF16)
            nc.vector.tensor_tensor(
                pe[:],
                cmp_tiles[blk // 2][:, :, boff : boff + BLK],
                ee[:][:, None, :].broadcast_to([P, C, BLK]),
                op=mybir.AluOpType.mult,
            )
            pe_tiles[blk] = pe

        emit_seg(nblk - 2)
        emit_seg(nblk - 1)

        # ---- export raw partial sums; host combines + divides ----
        res = opool.tile([out_rows, D + 1], F32)
        nc.scalar.activation(res[:], seg[:], mybir.ActivationFunctionType.Copy)
        nc.sync.dma_start(out.ap(), res[:])

    nc.compile()
    return nc


_PROGRAM_CACHE: dict = {}


def _get_program(N, mp, b2, b1_nonzero):
    key = (N, mp, float(b2), bool(b1_nonzero), PE_ENGINE, USE_FP8, USE_FP8P, SEG2)
    if key not in _PROGRAM_CACHE:
        _PROGRAM_CACHE[key] = _build_program(N, mp, b2, b1_nonzero)
    return _PROGRAM_CACHE[key]


def _host_prep(W1, b1, W2, b2):
    """Fold W2 into W1: scale columns by |w2|, positive-w2 columns first."""
    w2 = np.asarray(W2, np.float32).reshape(-1)
    b1 = np.asarray(b1, np.float32).reshape(-1)
    order = np.argsort(~(w2 >= 0), kind="stable")  # positives first
    mp = int((w2 >= 0).sum())
    w1s = (np.asarray(W1, np.float32)[:, order] * np.abs(w2[order])).astype(BF16_NP)
    b1s = (b1[order] * np.abs(w2[order])).astype(BF16_NP)[None, :]
    b1_nonzero = bool(np.any(b1 != 0))
    return w1s, b1s, mp, float(np.asarray(b2).reshape(-1)[0]), b1_nonzero


def kernel(features, cluster_assignments, W1, b1, W2, b2, num_clusters):
    global LAST_RESULTS
    features = np.asarray(features, np.float32)
    B, N, Din = features.shape
    assert Din == D
    assert int(num_clusters) == C

    w1s, b1s, mp, b2f, b1_nonzero = _host_prep(W1, b1, W2, b2)
    a = np.asarray(cluster_assignments).astype(np.int32)
    iota_np = np.ascontiguousarray(
        np.broadcast_to(
            np.arange(C, dtype=BF16_NP)[None, :, None], (P, C, 2 * BLK)
        )
    ).reshape(P, C * 2 * BLK)

    nc = _get_program(N, mp, b2f, b1_nonzero)

    J = N // P
    ft_np_dt = F8_NP if USE_FP8 else BF16_NP
    fp_np_dt = F8_NP if USE_FP8P else BF16_NP
    in_maps = []
    for b in range(B):
        fp32 = features[b].reshape(P, J, D)  # t = p*J + j
        featp_np = np.empty((P, J, D + 1), fp_np_dt)
        featp_np[:, :, :D] = fp32.astype(fp_np_dt)
        featp_np[:, :, D] = fp_np_dt(1.0)
        featt_np = np.ascontiguousarray(fp32.transpose(2, 1, 0)).astype(ft_np_dt)
        in_maps.append(
            {
                "featp": featp_np.reshape(P, J * (D + 1)),
                "featt": featt_np.reshape(D, J * P),
                "a_bf": a[b].reshape(P, J).astype(BF16_NP),
                "iota_rep": iota_np,
                "w1s": w1s,
                "b1s": b1s,
            }
        )

    res = run_bass_kernel_spmd(nc, in_maps, list(range(B)))
    LAST_RESULTS = res
    raw = np.stack(
        [np.asarray(res.results[i]["out"], np.float64) for i in range(B)], axis=0
    )
    if SEG2:
        raw = raw[:, :C, :] + raw[:, C:, :]
    numer = raw[:, :, :D]
    denom = np.maximum(raw[:, :, D:], 1e-20)
    return (numer / denom).astype(np.float32)
